# revision 31
# baseline (speedup 1.0000x reference)
"""Trainium2 Bass kernel for nn_DecoderBlock (dense transformer decoder block).

Sharding: batch x sequence-half across 8 cores, zero collectives.
  core c -> batch b = c // 2, query rows [ (c%2)*512, (c%2)*512+512 ).
K/V are computed per-core from the full 1024-token inputs (duplicated across
the pair of cores sharing a batch).

Causal mask without per-core control flow:
  - self-attn keys are host-reordered per core as [diag 512 block | far 512 block]
  - diag block gets a constant triangular additive mask (same on all cores)
  - far block gets a per-core scalar bias (0 or -30000) folded into exp's bias
    (h=0's far block is dummy zero tokens, fully masked).

Layouts: activations feature-major [D, tokens]; weights natural [in, out] as
stationary matmul operands; attention scores computed transposed [k, q] so the
softmax denominator (sum over k = partitions) comes from a PE ones-column and
probs @ V needs no transposes.

Matmuls in float32r (full speed at N>=512); Q/K/probs/V in bf16.
Scores for each head pair are emitted interleaved so the two K=64 matmuls
land in disjoint PE row-groups and run concurrently on hardware.
"""

import sys
import numpy as np

sys.path.insert(0, "/opt/trn_rl_repo")

D = 1024
H = 16
HD = 64
FF = 4096
NT = 1024   # kv tokens
TQ = 512    # query tokens per core
NEG = -30000.0
SCALE = 1.0 / 32.0  # 1/sqrt(D)
N_CORES = 8

_CACHE = {}


def _build():
    import concourse.bass as bass
    import concourse.tile as tile
    from concourse import bacc, mybir
    from contextlib import ExitStack

    f32 = mybir.dt.float32
    f32r = mybir.dt.float32r
    bf16 = mybir.dt.bfloat16
    AF = mybir.ActivationFunctionType
    OP = mybir.AluOpType

    nc = bacc.Bacc("TRN2", target_bir_lowering=False, debug=False,
                   num_devices=N_CORES)

    def inp(name, shape, dt=None):
        return nc.declare_dram_parameter(name, list(shape), dt or f32,
                                         isOutput=False)

    xqT_d = inp("xqT", (D, TQ), f32r)
    xkvT_d = inp("xkvT", (D, NT), f32r)
    encT_d = inp("encT", (D, NT), f32r)
    triT_d = inp("triT", (TQ, TQ), bf16)
    pack_d = inp("packP", (128, 89))
    ones_d = inp("onesP", (128, 128), f32r)
    Wq1_d = inp("Wq1", (D, D), f32r)
    Wk1_d = inp("Wk1", (D, D), f32r)
    Wv1_d = inp("Wv1", (D, D), f32r)
    Wq2_d = inp("Wq2", (D, D), f32r)
    Wk2_d = inp("Wk2", (D, D), f32r)
    Wv2_d = inp("Wv2", (D, D), f32r)
    Wf1_d = inp("W_ff1", (D, FF), f32r)
    Wf2_d = inp("W_ff2", (FF, D), f32r)
    outT_d = nc.declare_dram_parameter("outT", [D, TQ], f32, isOutput=True)

    with ExitStack() as ctx:
        ctx.enter_context(nc.allow_low_precision(reason="fp32r matmul pipeline"))
        tc = ctx.enter_context(tile.TileContext(nc))
        pers = ctx.enter_context(tc.tile_pool(name="pers", bufs=1))
        kvp = ctx.enter_context(tc.tile_pool(name="kvp", bufs=1))
        act = ctx.enter_context(tc.tile_pool(name="act", bufs=1))
        wpool = ctx.enter_context(tc.tile_pool(name="wpool", bufs=2))
        expool = ctx.enter_context(tc.tile_pool(name="expool", bufs=2))
        tmpp = ctx.enter_context(tc.tile_pool(name="tmpp", bufs=2))
        psb = ctx.enter_context(tc.tile_pool(name="psb", bufs=2, space="PSUM"))
        ps = ctx.enter_context(tc.tile_pool(name="ps", bufs=2, space="PSUM"))
        psx = ctx.enter_context(tc.tile_pool(name="psx", bufs=2, space="PSUM"))

        mm = nc.tensor.matmul
        dma = nc.sync.dma_start
        TT = nc.vector.tensor_tensor
        STT = nc.vector.scalar_tensor_tensor

        def load_wbig(w_dram, r0, r1, c0, c1):
            """DMA W[r0:r1, c0:c1] into an SBUF tile [128, kt, c1-c0], one DMA
            per 128-row slab so consumers can start after the first slab."""
            kt = (r1 - r0) // 128
            t = wpool.tile([128, kt, c1 - c0], f32r, tag="wbig")
            dma(out=t, in_=w_dram[r0:r1, c0:c1].rearrange("(t p) n -> p t n",
                                                          p=128))
            return t

        def proj_chunks(w_dram, rhs3, out3, n_tok):
            """Emission chunks for out3 = W.T @ rhs (feature-major)."""
            nn = n_tok // 512
            chunks = []
            state = {}
            for mb in range(4):
                def load(mb=mb):
                    state[mb] = load_wbig(w_dram, 0, D, mb * 256, (mb + 1) * 256)
                chunks.append(load)
                for mi in range(2):
                    for n in range(nn):
                        def chain(mb=mb, mi=mi, n=n):
                            w = state[mb]
                            m = mb * 2 + mi
                            p = ps.tile([128, 512], f32, tag="mm")
                            for k in range(8):
                                mm(out=p,
                                   lhsT=w[:, k, mi * 128:(mi + 1) * 128],
                                   rhs=rhs3[:, k, n * 512:(n + 1) * 512],
                                   start=(k == 0), stop=(k == 7))
                            nc.vector.tensor_copy(
                                out=out3[:, m, n * 512:(n + 1) * 512], in_=p)
                        chunks.append(chain)
            return chunks

        def vproj_chunks(w_dram, rhs3, vaug3):
            """Emission chunks for the token-major V projection (ones interleaved).

            vaug3: [128, 8, 16*65] bf16; head h cols [65h, 65h+64), ones at 65h+64.
            """
            chunks = []
            state = {}
            for n in range(4):
                def load(n=n):
                    state[n] = load_wbig(w_dram, 0, D, n * 256, (n + 1) * 256)
                chunks.append(load)
                for mt in range(8):
                    def chain(n=n, mt=mt):
                        wv = state[n]
                        p = ps.tile([128, 256], f32, tag="mm")
                        for k in range(8):
                            mm(out=p,
                               lhsT=rhs3[:, k, mt * 128:(mt + 1) * 128],
                               rhs=wv[:, k, :], start=(k == 0), stop=(k == 7))
                        dst = vaug3[:, mt, 260 * n:260 * n + 260].rearrange(
                            "p (h e) -> p h e", e=65)[:, :, 0:64]
                        nc.vector.tensor_copy(
                            out=dst, in_=p.rearrange("p (h e) -> p h e", e=64))
                    chunks.append(chain)
            return chunks

        def emit(chunks, n=None):
            for _ in range(len(chunks) if n is None else n):
                if chunks:
                    chunks.pop(0)()

        def proj_ffmajor(w_dram, rhs3, out3, n_tok):
            emit(proj_chunks(w_dram, rhs3, out3, n_tok))

        def vproj(w_dram, rhs3, vaug3):
            emit(vproj_chunks(w_dram, rhs3, vaug3))

        def attention(q3, k3, vaug3, masked, attn3, resid3, bg=None):
            """scoresT attention; writes attn3 = ctx*R + resid3 (feature-major).

            Processes head pairs (2t, 2t+1); their K=64 score matmuls are
            emitted adjacently with disjoint row-groups (partition bases 0/64)
            for PE concurrency; score chunks are paired into [128, 2, 512]
            psum tiles so mask-add and exp run at FD=1024. `bg` chunks
            (independent PE work) are interleaved between head pairs to fill
            the ACT-bound stretches.
            """
            bg = bg or []
            per_ft = (len(bg) + 7) // 8 if bg else 0
            for ft in range(8):
                emit(bg, per_ft)
                ex_a = expool.tile([128, 8, TQ], bf16, tag="ex")
                ex_b = expool.tile([128, 8, TQ], bf16, tag="ex")
                exs = [ex_a, ex_b]
                for kp in range(4):
                    sc_a = psb.tile([128, 2, TQ], f32, tag="big2")
                    sc_b = psb.tile([128, 2, TQ], f32, tag="big2")
                    scs = [sc_a, sc_b]
                    for kc2 in range(2):
                        kc = 2 * kp + kc2
                        for j, fo in ((0, 0), (1, 64)):
                            mm(out=scs[j][:, kc2, :],
                               lhsT=k3[fo:fo + 64, ft, kc * 128:(kc + 1) * 128],
                               rhs=q3[fo:fo + 64, ft, :],
                               start=True, stop=True)
                    bias = 0.0
                    if masked and kp < 2:
                        for j in range(2):
                            TT(out=scs[j], in0=scs[j],
                               in1=tri_sb[:, 2 * kp:2 * kp + 2, :], op=OP.add)
                    if masked and kp >= 2:
                        bias = bfar_sb
                    for j in range(2):
                        nc.scalar.activation(
                            out=exs[j][:, 2 * kp:2 * kp + 2, :], in_=scs[j],
                            func=AF.Exp, bias=bias, scale=SCALE)
                for j in range(2):
                    h = 2 * ft + j
                    fo = j * 64
                    cx = psx.tile([65, TQ], f32, tag="cxs")
                    for kc in range(8):
                        mm(out=cx, lhsT=vaug3[:, kc, 65 * h:65 * h + 65],
                           rhs=exs[j][:, kc, :], start=(kc == 0), stop=(kc == 7))
                    rec = tmpp.tile([65, TQ], f32r, tag="rec")
                    nc.vector.reciprocal(out=rec[64:65, :], in_=cx[64:65, :])
                    Rp = psx.tile([64, TQ], f32, tag="cxs")
                    mm(out=Rp, lhsT=ones65[64:65, 0:64], rhs=rec[64:65, :],
                       start=True, stop=True)
                    Rs = tmpp.tile([64, TQ], f32, tag="rsb")
                    nc.vector.tensor_copy(out=Rs, in_=Rp)
                    if fo == 0:
                        TT(out=attn3[0:64, ft, :], in0=cx[0:64, :], in1=Rs,
                           op=OP.mult)
                    else:
                        ctmp = tmpp.tile([64, TQ], f32r, tag="rec")
                        TT(out=ctmp, in0=cx[0:64, :], in1=Rs, op=OP.mult)
                        dma(out=attn3[64:128, ft, :], in_=ctmp)
            emit(bg)
            for t in range(8):
                TT(out=attn3[:, t, :], in0=attn3[:, t, :], in1=resid3[:, t, :],
                   op=OP.add)

        def layernorm(s3, gcol, bcol, out3):
            """out3 = (s3 - mean)/sqrt(var+eps) * g + b, reducing over features
            (partition axis across the 8 f-tiles), per token."""
            s1 = psx.tile([1, TQ], f32, tag="cxs")
            s2 = psx.tile([1, TQ], f32, tag="cxs")
            for t in range(8):
                mm(out=s1, lhsT=ones_col, rhs=s3[:, t, :],
                   start=(t == 0), stop=(t == 7))
                sq = tmpp.tile([128, TQ], f32r, tag="lnt")
                nc.scalar.square(sq, s3[:, t, :])
                mm(out=s2, lhsT=ones_col, rhs=sq,
                   start=(t == 0), stop=(t == 7))
            nmean = tmpp.tile([1, TQ], f32r, tag="sm")
            nc.scalar.activation(out=nmean, in_=s1, func=AF.Copy, scale=-1.0 / D)
            nm_ps = psb.tile([128, TQ], f32, tag="big2")
            mm(out=nm_ps, lhsT=ones_row[0:1, :], rhs=nmean,
               start=True, stop=True)
            exsq = tmpp.tile([1, TQ], f32r, tag="sm")
            nc.scalar.activation(out=exsq, in_=s2, func=AF.Copy, scale=1.0 / D)
            msq = tmpp.tile([1, TQ], f32, tag="lnt")
            nc.scalar.square(msq, nmean)
            # exsq -> var -> std -> rstd, all in place
            TT(out=exsq, in0=exsq, in1=msq, op=OP.subtract)
            nc.scalar.activation(out=exsq, in_=exsq, func=AF.Sqrt,
                                 bias=eps_sb[0:1, 0:1], scale=1.0)
            nc.vector.reciprocal(out=exsq, in_=exsq)
            rs_ps = psb.tile([128, TQ], f32, tag="big2")
            mm(out=rs_ps, lhsT=ones_row[0:1, :], rhs=exsq,
               start=True, stop=True)
            for t in range(8):
                t1 = tmpp.tile([128, TQ], f32, tag="lnt")
                STT(out=t1, in0=s3[:, t, :], scalar=1.0, in1=nm_ps,
                    op0=OP.bypass, op1=OP.add)
                STT(out=t1, in0=t1, scalar=gcol[:, t:t + 1], in1=rs_ps,
                    op0=OP.mult, op1=OP.mult)
                nc.vector.tensor_scalar_add(out=out3[:, t, :], in0=t1,
                                            scalar1=bcol[:, t:t + 1])

        # =================== stage A: causal self-attention ===================
        # Emission order front-loads the critical path: xq + Wq1 first so the
        # PE starts as soon as the first weight slab lands.
        xq = act.tile([128, 8, TQ], f32r, tag="xq")
        dma(out=xq, in_=xqT_d[:, :].rearrange("(t p) q -> p t q", p=128))
        q1 = act.tile([128, 8, TQ], bf16, tag="q")
        q1c = proj_chunks(Wq1_d, xq, q1, TQ)
        emit(q1c, 3)  # first weight block + two chains, then kick off xkv DMA

        xkv = kvp.tile([128, 8, NT], f32r, tag="kv")
        dma(out=xkv, in_=xkvT_d[:, :].rearrange("(t p) q -> p t q", p=128))
        emit(q1c)
        k1 = act.tile([128, 8, NT], bf16, tag="k", bufs=2)
        proj_ffmajor(Wk1_d, xkv, k1, NT)
        v1 = act.tile([128, 8, 16 * 65], bf16, tag="vh", bufs=2)
        nc.vector.memset(
            v1.rearrange("p c (h e) -> p c h e", e=65)[:, :, :, 64:65], 1.0)
        vproj(Wv1_d, xkv, v1)

        # ---------- constants (DMAs deferred past the hot startup path) ------
        tri_sb = pers.tile([128, 4, TQ], bf16, tag="tri")
        dma(out=tri_sb, in_=triT_d[:, :].rearrange("(c p) q -> p c q", p=128))
        pack_sb = pers.tile([128, 89], f32, tag="pack")
        dma(out=pack_sb, in_=pack_d[:, :])
        gb = {}
        for i_, nm in enumerate(("g1", "b1", "g2", "b2", "g3", "b3")):
            gb[nm] = pack_sb[:, i_ * 8:(i_ + 1) * 8]
        bf1_sb = pack_sb[:, 48:80]
        bf2_sb = pack_sb[:, 80:88]
        bfar_sb = pack_sb[:, 88:89]
        ones_row = pers.tile([1, 128], f32r, tag="ones_row")
        dma(out=ones_row, in_=ones_d[0:1, :])
        ones65 = pers.tile([65, 128], f32r, tag="ones65")
        dma(out=ones65, in_=ones_d[0:65, :])
        ones_col = pers.tile([128, 1], f32r, tag="ones_col")
        dma(out=ones_col, in_=ones_d[:, 0:1])
        eps_sb = pers.tile([1, 1], f32, tag="eps")
        nc.vector.memset(eps_sb, 1e-5)

        # Stage-B K/V projections depend only on encoder_out, so they are
        # emitted as background chunks interleaved into attention A (which is
        # otherwise ACT-bound on the exp evaluations).
        enc = kvp.tile([128, 8, NT], f32r, tag="kv")
        dma(out=enc, in_=encT_d[:, :].rearrange("(t p) q -> p t q", p=128))
        k2 = act.tile([128, 8, NT], bf16, tag="k", bufs=2)
        v2 = act.tile([128, 8, 16 * 65], bf16, tag="vh", bufs=2)
        nc.vector.memset(
            v2.rearrange("p c (h e) -> p c h e", e=65)[:, :, :, 64:65], 1.0)
        bgA = proj_chunks(Wk2_d, enc, k2, NT) + vproj_chunks(Wv2_d, enc, v2)

        au1 = act.tile([128, 8, TQ], f32r, tag="au")
        attention(q1, k1, v1, True, au1, xq, bg=bgA)
        z1 = act.tile([128, 8, TQ], f32r, tag="z")
        layernorm(au1, gb["g1"], gb["b1"], z1)

        # =================== stage B: cross-attention ===================
        # Q2 m-tiles stream in as background chunks: head pair ft only needs
        # Q2's m-tile ft, so the projection overlaps the attention itself.
        q2 = act.tile([128, 8, TQ], bf16, tag="q")
        bgB = proj_chunks(Wq2_d, z1, q2, TQ)
        emit(bgB, 3)
        au2 = act.tile([128, 8, TQ], f32r, tag="au")
        attention(q2, k2, v2, False, au2, z1, bg=bgB)
        z2 = act.tile([128, 8, TQ], f32r, tag="xq")
        layernorm(au2, gb["g2"], gb["b2"], z2)

        # =================== stage C: FFN (4 chunks of 1024 ff dims) ==========
        s3 = act.tile([128, 8, TQ], f32r, tag="au")
        for rep in range(4):
            hsb = act.tile([128, 8, TQ], f32r, tag="vh", bufs=2)
            for mg in range(4):
                wf1 = load_wbig(Wf1_d, 0, D,
                                rep * 1024 + mg * 256, rep * 1024 + mg * 256 + 256)
                for j in range(2):
                    idx = rep * 8 + mg * 2 + j
                    p = ps.tile([128, TQ], f32, tag="mm")
                    for k in range(8):
                        mm(out=p, lhsT=wf1[:, k, j * 128:(j + 1) * 128],
                           rhs=z2[:, k, :], start=(k == 0), stop=(k == 7))
                    nc.scalar.activation(out=hsb[:, mg * 2 + j, :], in_=p,
                                         func=AF.Gelu,
                                         bias=bf1_sb[:, idx:idx + 1], scale=1.0)
            for mp in range(4):
                wf2 = load_wbig(Wf2_d, rep * 1024, rep * 1024 + 1024,
                                mp * 256, mp * 256 + 256)
                for m2 in range(2):
                    m = mp * 2 + m2
                    p = ps.tile([128, TQ], f32, tag="mm")
                    for k in range(8):
                        mm(out=p, lhsT=wf2[:, k, m2 * 128:(m2 + 1) * 128],
                           rhs=hsb[:, k, :], start=(k == 0), stop=(k == 7))
                    if rep == 0:
                        TT(out=s3[:, m, :], in0=p, in1=z2[:, m, :], op=OP.add)
                    else:
                        TT(out=s3[:, m, :], in0=p, in1=s3[:, m, :], op=OP.add)
        for m in range(8):
            nc.vector.tensor_scalar_add(out=s3[:, m, :], in0=s3[:, m, :],
                                        scalar1=bf2_sb[:, m:m + 1])

        z3 = act.tile([128, 8, TQ], f32, tag="z")
        layernorm(s3, gb["g3"], gb["b3"], z3)
        dma(out=outT_d[:, :].rearrange("(t p) q -> p t q", p=128), in_=z3)

    nc.compile()
    return nc


def _get_nc():
    if "nc" not in _CACHE:
        _CACHE["nc"] = _build()
    return _CACHE["nc"]


def make_in_maps(x, encoder_out, Wq1, Wk1, Wv1, g1, b1, Wq2, Wk2, Wv2, g2, b2,
                 W_ff1, b_ff1, W_ff2, b_ff2, g3, b3):
    """Build the 8 per-core input dicts from full inputs."""
    f = np.float32
    import ml_dtypes
    tri = np.where(np.arange(TQ)[:, None] <= np.arange(TQ)[None, :], 0.0,
                   NEG).astype(ml_dtypes.bfloat16)
    def col8(v):
        return np.reshape(np.asarray(v, f), (8, 128)).T
    packP = np.concatenate(
        [col8(g1), col8(b1), col8(g2), col8(b2), col8(g3), col8(b3),
         np.reshape(np.asarray(b_ff1, f), (32, 128)).T,
         col8(b_ff2), np.zeros((128, 1), f)], axis=1)
    shared = {
        "triT": tri,
        "onesP": np.ones((128, 128), f),
        "packP": np.ascontiguousarray(packP),
        "Wq1": np.ascontiguousarray(Wq1, f), "Wk1": np.ascontiguousarray(Wk1, f),
        "Wv1": np.ascontiguousarray(Wv1, f), "Wq2": np.ascontiguousarray(Wq2, f),
        "Wk2": np.ascontiguousarray(Wk2, f), "Wv2": np.ascontiguousarray(Wv2, f),
        "W_ff1": np.ascontiguousarray(W_ff1, f),
        "W_ff2": np.ascontiguousarray(W_ff2, f),
    }
    in_maps = []
    for c in range(N_CORES):
        b, h = c // 2, c % 2
        q0 = h * TQ
        xb = np.asarray(x[b], f)          # [1024, 1024] token-major
        xqT = np.ascontiguousarray(xb[q0:q0 + TQ, :].T)
        diag = xb[q0:q0 + TQ, :]
        far = xb[0:TQ, :] if h == 1 else np.zeros((TQ, D), f)
        xkvT = np.ascontiguousarray(np.concatenate([diag, far], axis=0).T)
        encT = np.ascontiguousarray(np.asarray(encoder_out[b], f).T)
        m = dict(shared)
        pc = packP.copy()
        pc[:, 88] = 0.0 if h == 1 else NEG
        m.update({"xqT": xqT, "xkvT": xkvT, "encT": encT,
                  "packP": np.ascontiguousarray(pc)})
        in_maps.append(m)
    return in_maps


def run_cores(in_maps, trace=False, **kw):
    from concourse.bass_utils import run_bass_kernel_spmd
    nc = _get_nc()
    return run_bass_kernel_spmd(nc, in_maps, list(range(N_CORES)), trace=trace,
                                **kw)


def kernel(**inputs):
    in_maps = make_in_maps(**inputs)
    res = run_cores(in_maps).results
    BS = inputs["x"].shape[0]
    y = np.empty((BS, NT, D), np.float32)
    for c in range(N_CORES):
        b, h = c // 2, c % 2
        q0 = h * TQ
        y[b, q0:q0 + TQ, :] = res[c]["outT"].T
    return y
